# revision 19
# baseline (speedup 1.0000x reference)
"""Trainium2 Bass kernel for the dual-attention module (spatial + channel attention).

Contract: kernel(**inputs) takes the FULL inputs (x: (16,1024,64,64) f32 plus four
1x1-conv weight matrices) and returns the FULL output (16,1024,64,64) f32.
Internally shards data-parallel over batch across 8 NeuronCores (2 samples/core),
weights replicated.

Per-sample math (b, c=1024, ch=512, hw=4096):
  conv(w) = relu(w @ X)               X = x[b] as (1024, 4096)
  mask    = softmax(conv(w_qr))       over hw          (spatial attn branch)
  ctx     = conv(w_vr) @ mask         (ch,)
  s       = sigmoid(layernorm(ctx))   (ch,)
  avg     = softmax(mean_hw(conv(w_ql)))               (channel attn branch)
  chan    = sigmoid(avg @ conv(w_vl)) (hw,)
  out[0:512]    = x * (1 + s*chan)                     ("sequence")
  out[512:1024] = x * (1 + s + chan)                   ("parallel")

Kernel strategy per core (mixed bf16 / fp8-DoubleRow convs):
  - On TRN2 every 512-col matmul streams at ~219ns regardless of dtype, so
    instruction COUNT is what matters; fp8 DoubleRow contracts 2 k-tiles per
    instruction (2x instruction efficiency), bf16 contracts 1.
  - vr (context branch, layernorm-amplified) runs in bf16 from the xb tiles
    that are resident anyway for the finale multiply: same instruction count
    as residually-compensated fp8 but near-exact.  qr (mask logits, also
    LN-amplified but a single output row) runs uncompensated fp8 DoubleRow:
    its ~5e-3 error fits the budget freed by the bf16 vr.  ql/vl stay plain
    fp8 DoubleRow (their error washes out through mean(4096)/sum(512) and
    sigmoids).  Scale SC=256 on the fp8 weights folds out via exp scales.
  - x is loaded twice: fp8 (conv rhs, 4MB/sample) and bf16 (finale multiply,
    8MB/sample).  Output is stored bf16 (8MB/sample) and upconverted on the
    host; bf16 keeps elementwise error ~4e-3 << the 2e-2 gate.
  - Softmaxes are computed unnormalized (exp only); 1/Z folded into later
    scalar multiplies.  qr weights are column-replicated so mask psums arrive
    broadcast across partitions; the channel contraction uses a replicated
    fp32r lhsT so chan arrives pre-broadcast.
  - Cross-partition reductions (LN stats, channel-softmax Z) via exact-f32
    gpsimd.partition_all_reduce.
  - Phase C software-pipelines the vl conv one chunk ahead of the chan
    contraction so the PE never waits on the ACT relu or the post-B stats.
  - Finale ops are bf16 end-to-end (DVE 2x mode) and spread across
    Pool/DVE/ACT; stores on the Sync DGE with next-sample loads emitted ahead.
"""

import sys

sys.path.insert(0, "/opt/trn_rl_repo")

import numpy as np

import concourse.bass as bass  # noqa: F401  (bass must import before bacc)
import concourse.tile as tile
from concourse import bacc, bass_isa, bass_utils, mybir

# Problem constants (hardcoded per contract).
B, C, H, W = 16, 1024, 64, 64
HW = H * W               # 4096
CH = C // 2              # 512
N_CORES = 8
S = B // N_CORES         # 2 samples per core
P = 128                  # SBUF partitions
KT = C // P              # 8 k-tiles over input channels
MT = CH // P             # 4 m-tiles over output channels
NW = 512                 # n-chunk width (one PSUM bank of f32)
NCH = HW // NW           # 8 n-chunks
LN_EPS = 1e-5
SC = 256.0               # fp8 weight scale (folded out through exp scales / LN)

F32 = mybir.dt.float32
F32R = mybir.dt.float32r
BF16 = mybir.dt.bfloat16
F8 = mybir.dt.float8e4
Alu = mybir.AluOpType
Act = mybir.ActivationFunctionType
AxX = mybir.AxisListType.X
DR = mybir.MatmulPerfMode.DoubleRow

_cache = {}


def _build():
    nc = bacc.Bacc(
        "TRN2",
        target_bir_lowering=False,
        debug=False,
        num_devices=N_CORES,
        dynamic_dma_scratch_size=512,
    )

    # x: [S, chunk, P, KT, NW] so one chunk is a single DMA with contiguous
    # per-partition bytes (4KB fp8 / 8KB bf16); weights partition-major.
    xq_d = nc.dram_tensor("xq", [S, NCH, P, KT, NW], F8, kind="ExternalInput")
    xb_d = nc.dram_tensor("xb", [S, NCH, P, KT, NW], BF16, kind="ExternalInput")
    wqr8_d = nc.dram_tensor("wqr8", [P, KT, P], F8, kind="ExternalInput")
    wvrb_d = nc.dram_tensor("wvrb", [P, KT, CH], BF16, kind="ExternalInput")
    wql_d = nc.dram_tensor("wql", [P, KT, CH], F8, kind="ExternalInput")
    wvl_d = nc.dram_tensor("wvl", [P, KT, CH], F8, kind="ExternalInput")
    # out is [S, P, KT, HW] so a whole chunk stores as ONE dma whose dram AP
    # dims (P, KT, NW) match the sbuf tile; host transposes (cheap view).
    out_d = nc.dram_tensor("out", [S, P, KT, HW], BF16, kind="ExternalOutput")

    with tile.TileContext(nc) as tc:
        with (
            tc.tile_pool(name="xqp", bufs=2 * NCH) as xqp,
            tc.tile_pool(name="xbp", bufs=NCH + 2) as xbp,
            tc.tile_pool(name="wp", bufs=1) as wp,
            tc.tile_pool(name="actp", bufs=2) as actp,
            tc.tile_pool(name="deadp", bufs=1) as deadp,
            tc.tile_pool(name="thp", bufs=2) as thp,
            tc.tile_pool(name="smp", bufs=2) as smp,
            tc.tile_pool(name="erp", bufs=2 * MT) as erp,
            tc.tile_pool(name="chp", bufs=3) as chp,
            tc.tile_pool(name="tp", bufs=10) as tp,
            tc.tile_pool(name="otp", bufs=3) as otp,
            tc.tile_pool(name="psA", bufs=2, space="PSUM") as psA,
            tc.tile_pool(name="psB", bufs=6, space="PSUM") as psB,
        ):
            # ---- constants ----
            epst = wp.tile([P, 1], F32, name="epst", tag="epst")
            nc.vector.memset(epst[:], LN_EPS)

            # ---- weight tiles: one DMA per tensor, emitted in first-use
            # priority order interleaved with the sample-0 x loads ----
            wqr8_sb = wp.tile([P, KT, P], F8, name="wqr8sb", tag="wqr8sb")
            wvrb_sb = wp.tile([P, KT, CH], BF16, name="wvrbsb", tag="wvrbsb")
            wql_sb = wp.tile([P, KT, CH], F8, name="wqlsb", tag="wqlsb")
            wvl_sb = wp.tile([P, KT, CH], F8, name="wvlsb", tag="wvlsb")

            def emit_xq_load(s_, i_):
                t = xqp.tile([P, KT, NW], F8, name=f"xq{s_}_{i_}", tag="xq")
                nc.sync.dma_start(t[:], xq_d.ap()[s_, i_])
                return t

            def emit_xb_load(s_, i_):
                t = xbp.tile([P, KT, NW], BF16, name=f"xb{s_}_{i_}", tag="xb")
                nc.sync.dma_start(t[:], xb_d.ap()[s_, i_])
                return t

            # sample-0 loads up front, ordered so A(0) can start ASAP:
            # qr needs wqr8+xq(0,0); vr needs wvrb+xb(0,0); then stream the
            # rest chunk-by-chunk (A consumes xq(i)+xb(i) per chunk).
            nc.sync.dma_start(wqr8_sb[:], wqr8_d.ap()[:])
            xq_all = {0: [], 1: []}
            xb_all = {0: [], 1: []}
            xq_all[0].append(emit_xq_load(0, 0))
            nc.sync.dma_start(wvrb_sb[:], wvrb_d.ap()[:])
            xb_all[0].append(emit_xb_load(0, 0))
            for i in range(1, NCH):
                xq_all[0].append(emit_xq_load(0, i))
                xb_all[0].append(emit_xb_load(0, i))
                if i == 1:
                    nc.sync.dma_start(wql_sb[:], wql_d.ap()[:])
                elif i == 2:
                    nc.sync.dma_start(wvl_sb[:], wvl_d.ap()[:])

            def comp_mm(ps, w8, wr, xq, m, nk=2):
                """Residual-compensated DoubleRow conv into psum group.

                w8/wr: [P, KT, cols] fp8 tiles (wr=None for uncompensated);
                xq: [P, KT, NW] fp8; m: output m-tile index (cols slice).
                """
                lo, hi = m * P, (m + 1) * P
                wts = [w8] if wr is None else [w8, wr]
                n = len(wts) * (KT // nk)
                j = 0
                for wt in wts:
                    for a in range(KT // nk):
                        nc.tensor.matmul(
                            ps[:],
                            wt[:, nk * a : nk * a + nk, lo:hi],
                            xq[:, nk * a : nk * a + nk, :],
                            start=(j == 0), stop=(j == n - 1),
                            perf_mode=DR,
                        )
                        j += 1

            for s in range(S):
                xq_t = xq_all[s]

                # per-sample accumulators
                zpart = smp.tile([P, NCH], F32, name=f"zpart{s}", tag="zpart")
                ctxp = [
                    smp.tile([P, NCH], F32, name=f"ctxp{s}_{m}", tag=f"ctxp{m}")
                    for m in range(MT)
                ]
                gp = [
                    smp.tile([P, NCH], F32, name=f"gp{s}_{m}", tag=f"gp{m}")
                    for m in range(MT)
                ]

                # ---- phase A: qr conv (fp8 DR, uncompensated) + vr conv
                # (bf16 weights x bf16 xb tiles, near-exact) ----
                for i in range(NCH):
                    psq = psA.tile([P, NW], F32, name=f"psq{s}_{i}", tag="psA")
                    comp_mm(psq, wqr8_sb, None, xq_t[i], 0)
                    # exp(relu(z)) == max(exp(z), 1): ACT exp (1/SC unscale),
                    # then DVE in-place max with Z partials via accum
                    et = actp.tile([P, NW], F32, name=f"et{s}_{i}", tag="et")
                    nc.scalar.activation(et[:], psq[:], Act.Exp, scale=1.0 / SC)
                    nc.vector.tensor_scalar(
                        et[:], et[:], 1.0, 0.0, Alu.max, Alu.add,
                        accum_out=zpart[:, i : i + 1],
                    )
                    for m in range(MT):
                        psv = psB.tile([P, NW], F32, name=f"psv{s}a{i}_{m}", tag="psB")
                        lo, hi = m * P, (m + 1) * P
                        for a in range(KT):
                            nc.tensor.matmul(
                                psv[:],
                                wvrb_sb[:, a, lo:hi],
                                xb_all[s][i][:, a, :],
                                start=(a == 0), stop=(a == KT - 1),
                            )
                        # ctx partial: sum_n relu(vr) * exp(relu(qr))
                        scr = deadp.tile([P, NW], F32, name=f"sttscr{s}", tag="sttscr")
                        nc.vector.scalar_tensor_tensor(
                            scr[:], psv[:], 0.0, et[:], Alu.max, Alu.mult,
                            accum_out=ctxp[m][:, i : i + 1],
                        )

                # ---- finalize mask Z and context; layernorm stats ----
                Zt = smp.tile([P, 1], F32, name=f"Z{s}", tag="Z")
                nc.vector.tensor_reduce(Zt[:], zpart[:], AxX, Alu.add)
                rZ = smp.tile([P, 1], F32, name=f"rZ{s}", tag="rZ")
                nc.vector.reciprocal(rZ[:], Zt[:])
                ctx44 = smp.tile([P, MT], F32, name=f"ctx44{s}", tag="ctx44")
                for m in range(MT):
                    cred = smp.tile([P, 1], F32, name=f"cred{s}_{m}", tag="cred")
                    nc.vector.tensor_reduce(cred[:], ctxp[m][:], AxX, Alu.add)
                    nc.vector.tensor_scalar(
                        ctx44[:, m : m + 1], cred[:], rZ[:], None, Alu.mult
                    )
                lnsum = smp.tile([P, MT], F32, name=f"lnsum{s}", tag="lnsum")
                nc.gpsimd.partition_all_reduce(
                    lnsum[:], ctx44[:], P, bass_isa.ReduceOp.add
                )
                tot = smp.tile([P, 1], F32, name=f"tot{s}", tag="tot")
                nc.vector.tensor_reduce(tot[:], lnsum[:], AxX, Alu.add)
                mu = smp.tile([P, 1], F32, name=f"mu{s}", tag="mu")
                nc.vector.tensor_scalar(mu[:], tot[:], 1.0 / CH, None, Alu.mult)
                d44 = smp.tile([P, MT], F32, name=f"d44{s}", tag="d44")
                nc.vector.tensor_scalar(d44[:], ctx44[:], mu[:], None, Alu.subtract)
                d2 = smp.tile([P, MT], F32, name=f"d2{s}", tag="d2")
                nc.vector.tensor_tensor(d2[:], d44[:], d44[:], Alu.mult)
                vsum = smp.tile([P, MT], F32, name=f"vsum{s}", tag="vsum")
                nc.gpsimd.partition_all_reduce(
                    vsum[:], d2[:], P, bass_isa.ReduceOp.add
                )
                vtot = smp.tile([P, 1], F32, name=f"vtot{s}", tag="vtot")
                nc.vector.tensor_reduce(vtot[:], vsum[:], AxX, Alu.add)
                var = smp.tile([P, 1], F32, name=f"var{s}", tag="var")
                nc.vector.tensor_scalar(var[:], vtot[:], 1.0 / CH, None, Alu.mult)

                # ---- phase B: ql conv, plain fp8 DoubleRow ----
                for i in range(NCH):
                    if s + 1 < S and i < 4:
                        # next sample's fp8 loads early (bufs=16 -> fresh slots)
                        xq_all[s + 1].append(emit_xq_load(s + 1, 2 * i))
                        xq_all[s + 1].append(emit_xq_load(s + 1, 2 * i + 1))
                    for m in range(MT):
                        psv = psB.tile([P, NW], F32, name=f"psv{s}b{i}_{m}", tag="psB")
                        comp_mm(psv, wql_sb, None, xq_t[i], m)
                        # relu + accumulate mean partials; alternate engines
                        if m % 2 == 0:
                            scr = deadp.tile([P, NW], F32, name=f"qlscr{s}", tag="qlscr")
                            nc.scalar.activation(
                                scr[:], psv[:], Act.Relu, accum_out=gp[m][:, i : i + 1]
                            )
                        else:
                            scr2 = deadp.tile([P, NW], F32, name=f"sttscr{s}b", tag="sttscr")
                            nc.vector.tensor_scalar(
                                scr2[:], psv[:], 0.0, 0.0, Alu.max, Alu.add,
                                accum_out=gp[m][:, i : i + 1],
                            )

                # ---- channel softmax + LN finalize (overlaps C's vl convs) ----
                g44 = smp.tile([P, MT], F32, name=f"g44{s}", tag="g44")
                for m in range(MT):
                    nc.vector.tensor_reduce(g44[:, m : m + 1], gp[m][:], AxX, Alu.add)
                e44 = smp.tile([P, MT], F32, name=f"e44{s}", tag="e44")
                nc.scalar.activation(e44[:], g44[:], Act.Exp, scale=1.0 / (HW * SC))
                std = smp.tile([P, 1], F32, name=f"std{s}", tag="std")
                nc.scalar.activation(std[:], var[:], Act.Sqrt, bias=epst[:])
                rstd = smp.tile([P, 1], F32, name=f"rstd{s}", tag="rstd")
                nc.vector.reciprocal(rstd[:], std[:])
                spre = smp.tile([P, MT], F32, name=f"spre{s}", tag="spre")
                nc.vector.tensor_scalar(
                    spre[:], ctx44[:], mu[:], rstd[:], Alu.subtract, Alu.mult
                )
                s44 = smp.tile([P, MT], F32, name=f"s44{s}", tag="s44")
                nc.scalar.activation(s44[:], spre[:], Act.Sigmoid)
                sp44 = smp.tile([P, MT], F32, name=f"sp44{s}", tag="sp44")
                nc.vector.tensor_scalar(sp44[:], s44[:], 1.0, None, Alu.add)

                ze = smp.tile([P, MT], F32, name=f"ze{s}", tag="ze")
                nc.gpsimd.partition_all_reduce(ze[:], e44[:], P, bass_isa.ReduceOp.add)
                zet = smp.tile([P, 1], F32, name=f"zet{s}", tag="zet")
                nc.vector.tensor_reduce(zet[:], ze[:], AxX, Alu.add)
                rZc = smp.tile([P, 1], F32, name=f"rZc{s}", tag="rZc")
                nc.vector.reciprocal(rZc[:], zet[:])
                erep = []
                for m in range(MT):
                    er = erp.tile([P, P], BF16, name=f"erep{s}_{m}", tag="erep")
                    # 1/SC compensates the fp8 scaling of wvl
                    nc.vector.tensor_scalar(
                        er[:], e44[:, m : m + 1].broadcast_to([P, P]),
                        1.0 / SC, None, Alu.mult,
                    )
                    erep.append(er)

                # ---- phase C: vl conv (one chunk ahead) -> chan attn ->
                # finale + store.  vl fp8 DoubleRow, chan fp32r replicated ----
                th_t = {}

                def emit_vl(i):
                    ths = []
                    for m in range(MT):
                        psv = psB.tile([P, NW], F32, name=f"psv{s}c{i}_{m}", tag="psB")
                        comp_mm(psv, wvl_sb, None, xq_t[i], m)
                        th = thp.tile([P, NW], BF16, name=f"th{s}_{i}_{m}", tag=f"th{m}")
                        nc.scalar.activation(th[:], psv[:], Act.Relu)
                        ths.append(th)
                    th_t[i] = ths

                emit_vl(0)
                for i in range(NCH):
                    if i + 1 < NCH:
                        emit_vl(i + 1)
                    pschan = psA.tile([P, NW], F32, name=f"psc{s}_{i}", tag="psA")
                    for m in range(MT):
                        nc.tensor.matmul(
                            pschan[:], erep[m][:], th_t[i][m][:],
                            start=(m == 0), stop=(m == MT - 1),
                            skip_group_check=True,
                        )
                    del th_t[i]
                    chant = chp.tile([P, NW], BF16, name=f"ch{s}_{i}", tag="chant")
                    nc.scalar.activation(chant[:], pschan[:], Act.Sigmoid, scale=rZc[:])
                    if s + 1 < S:
                        xb_all[s + 1].append(emit_xb_load(s + 1, i))
                    # finale: seq rows k<4: x*(1 + s*chan); par rows: x*(chan+1+s).
                    # bf16 end-to-end.  The attn tiles t are TS ops (4x on DVE
                    # in bf16; STT has no fast mode so it is avoided): 3 on
                    # Pool, 3+1 on DVE, 1 on ACT (relu(chan+sp) == chan+sp
                    # since chan,s > 0).  All 8 multiplies are 2x TT on DVE.
                    ot = otp.tile([P, KT, NW], BF16, name=f"ot{s}_{i}", tag="ot")
                    for k in range(KT):
                        xf = xb_all[s][i][:, k, :]
                        t = tp.tile([P, NW], BF16, name=f"t{s}_{i}_{k}", tag="t")
                        if k < MT:
                            # t = 1 + s*chan on DVE (4x bf16 TS)
                            nc.vector.tensor_scalar(
                                t[:], chant[:], s44[:, k : k + 1], 1.0,
                                Alu.mult, Alu.add,
                            )
                        elif k < KT - 1:
                            # t = chan + (1+s) on Pool (off DVE's critical path)
                            nc.gpsimd.tensor_scalar(
                                t[:], chant[:], sp44[:, k - MT : k - MT + 1], None,
                                Alu.add,
                            )
                        else:
                            nc.scalar.activation(
                                t[:], chant[:], Act.Relu,
                                bias=sp44[:, k - MT : k - MT + 1],
                            )
                        nc.vector.tensor_tensor(ot[:, k, :], t[:], xf, Alu.mult)
                    # one store per chunk: sbuf [P][KT, NW] -> dram (P, KT, NW)
                    nc.sync.dma_start(
                        out_d.ap()[s, :, :, i * NW : (i + 1) * NW], ot[:]
                    )

    nc.compile()
    return nc


def _prep_inputs(x, w_qr, w_vr, w_ql, w_vl):
    import ml_dtypes

    f8 = np.dtype(ml_dtypes.float8_e4m3)
    bf16 = np.dtype(ml_dtypes.bfloat16)

    x = np.asarray(x, dtype=np.float32).reshape(B, C, HW)
    wts = {}

    def pack_w(w):
        # (out, in) -> [P, KT, out]: w_pk[p, k, o] = w[o, 128k + p]
        w = np.asarray(w, dtype=np.float32)
        return np.ascontiguousarray(w.T.reshape(KT, P, CH).transpose(1, 0, 2))

    def comp8(w):
        w8 = (w * SC).astype(f8)
        r8 = ((w - w8.astype(np.float32) / SC) * SC).astype(f8)
        return w8, r8

    wts["wvrb"] = pack_w(w_vr).astype(bf16)
    wts["wql"] = (pack_w(w_ql) * SC).astype(f8)
    wts["wvl"] = (pack_w(w_vl) * SC).astype(f8)
    q = np.asarray(w_qr, dtype=np.float32).reshape(KT, P).T  # [P, KT]
    qrep = np.ascontiguousarray(np.broadcast_to(q[:, :, None], (P, KT, P)))
    wts["wqr8"] = (qrep * SC).astype(f8)

    in_maps = []
    for c in range(N_CORES):
        m = dict(wts)
        # [S, chunk, P, KT, NW]: xs[s, i, p, k, n] = x[s, 128k+p, 512i+n]
        xs = np.ascontiguousarray(
            x[S * c : S * (c + 1)]
            .reshape(S, KT, P, NCH, NW)
            .transpose(0, 3, 2, 1, 4)
        )
        m["xq"] = xs.astype(f8)
        m["xb"] = xs.astype(bf16)
        in_maps.append(m)
    return in_maps


def _run(x, w_qr, w_vr, w_ql, w_vl, trace=False):
    if "nc" not in _cache:
        _cache["nc"] = _build()
    nc = _cache["nc"]
    in_maps = _prep_inputs(x, w_qr, w_vr, w_ql, w_vl)
    res = bass_utils.run_bass_kernel_spmd(
        nc, in_maps, core_ids=list(range(N_CORES)), trace=trace
    )
    out = np.empty((B, C, HW), np.float32)
    for c in range(N_CORES):
        out[S * c : S * (c + 1)] = (
            res.results[c]["out"]
            .reshape(S, P, KT, HW)
            .transpose(0, 2, 1, 3)
            .reshape(S, C, HW)
            .astype(np.float32)
        )
    return out.reshape(B, C, H, W), res


def kernel(x, w_qr, w_vr, w_ql, w_vl):
    out, _ = _run(x, w_qr, w_vr, w_ql, w_vl, trace=False)
    return out


# revision 24
# speedup vs baseline: 1.7513x; 1.7513x over previous
"""Trainium2 Bass kernel for the dual-attention module (spatial + channel attention).

Contract: kernel(**inputs) takes the FULL inputs (x: (16,1024,64,64) f32 plus four
1x1-conv weight matrices) and returns the FULL output (16,1024,64,64) f32.
Internally shards data-parallel over batch across 8 NeuronCores (2 samples/core),
weights replicated.

Per-sample math (b, c=1024, ch=512, hw=4096):
  conv(w) = relu(w @ X)               X = x[b] as (1024, 4096)
  mask    = softmax(conv(w_qr))       over hw          (spatial attn branch)
  ctx     = conv(w_vr) @ mask         (ch,)
  s       = sigmoid(layernorm(ctx))   (ch,)
  avg     = softmax(mean_hw(conv(w_ql)))               (channel attn branch)
  chan    = sigmoid(avg @ conv(w_vl)) (hw,)
  out[0:512]    = x * (1 + s*chan)                     ("sequence")
  out[512:1024] = x * (1 + s + chan)                   ("parallel")

Kernel strategy per core (mixed bf16 / fp8-DoubleRow convs):
  - On TRN2 every 512-col matmul streams at ~219ns regardless of dtype, so
    instruction COUNT is what matters; fp8 DoubleRow contracts 2 k-tiles per
    instruction (2x instruction efficiency), bf16 contracts 1.
  - vr (context branch, layernorm-amplified) runs in bf16 from the xb tiles
    that are resident anyway for the finale multiply: same instruction count
    as residually-compensated fp8 but near-exact.  qr (mask logits, also
    LN-amplified but a single output row) runs uncompensated fp8 DoubleRow:
    its ~5e-3 error fits the budget freed by the bf16 vr.  ql/vl stay plain
    fp8 DoubleRow (their error washes out through mean(4096)/sum(512) and
    sigmoids).  Scale SC=256 on the fp8 weights folds out via exp scales.
  - x is loaded twice: fp8 (conv rhs, 4MB/sample) and bf16 (finale multiply,
    8MB/sample).  Output is stored bf16 (8MB/sample) and upconverted on the
    host; bf16 keeps elementwise error ~4e-3 << the 2e-2 gate.
  - Softmaxes are computed unnormalized (exp only); 1/Z folded into later
    scalar multiplies.  qr weights are column-replicated so mask psums arrive
    broadcast across partitions; the channel contraction uses a replicated
    fp32r lhsT so chan arrives pre-broadcast.
  - Cross-partition reductions (LN stats, channel-softmax Z) via exact-f32
    gpsimd.partition_all_reduce.
  - Phase C software-pipelines the vl conv one chunk ahead of the chan
    contraction so the PE never waits on the ACT relu or the post-B stats.
  - Finale ops are bf16 end-to-end (DVE 2x mode) and spread across
    Pool/DVE/ACT; stores on the Sync DGE with next-sample loads emitted ahead.
"""

import sys

sys.path.insert(0, "/opt/trn_rl_repo")

import numpy as np

import concourse.bass as bass  # noqa: F401  (bass must import before bacc)
import concourse.tile as tile
from concourse import bacc, bass_isa, bass_utils, mybir

# Problem constants (hardcoded per contract).
B, C, H, W = 16, 1024, 64, 64
HW = H * W               # 4096
CH = C // 2              # 512
N_CORES = 8
S = B // N_CORES         # 2 samples per core
P = 128                  # SBUF partitions
KT = C // P              # 8 k-tiles over input channels
MT = CH // P             # 4 m-tiles over output channels
NW = 512                 # n-chunk width (one PSUM bank of f32)
NCH = HW // NW           # 8 n-chunks
LN_EPS = 1e-5
SC = 256.0               # fp8 weight scale (folded out through exp scales / LN)

F32 = mybir.dt.float32
F32R = mybir.dt.float32r
BF16 = mybir.dt.bfloat16
F8 = mybir.dt.float8e4
Alu = mybir.AluOpType
Act = mybir.ActivationFunctionType
AxX = mybir.AxisListType.X
DR = mybir.MatmulPerfMode.DoubleRow

_cache = {}


def _build():
    nc = bacc.Bacc(
        "TRN2",
        target_bir_lowering=False,
        debug=False,
        num_devices=N_CORES,
        dynamic_dma_scratch_size=512,
    )

    # x: [S, chunk, P, KT, NW] so one chunk is a single DMA with contiguous
    # per-partition bytes (4KB fp8 / 8KB bf16); weights partition-major.
    xq_d = nc.dram_tensor("xq", [S, NCH, P, KT, NW], F8, kind="ExternalInput")
    xb_d = nc.dram_tensor("xb", [S, NCH, P, KT, NW], BF16, kind="ExternalInput")
    wqr8_d = nc.dram_tensor("wqr8", [P, KT, P], F8, kind="ExternalInput")
    wvrb_d = nc.dram_tensor("wvrb", [P, KT, CH], BF16, kind="ExternalInput")
    wql_d = nc.dram_tensor("wql", [P, KT, CH], F8, kind="ExternalInput")
    wvl_d = nc.dram_tensor("wvl", [P, KT, CH], F8, kind="ExternalInput")
    # out is [S, P, KT, HW] so a whole chunk stores as ONE dma whose dram AP
    # dims (P, KT, NW) match the sbuf tile; host transposes (cheap view).
    out_d = nc.dram_tensor("out", [S, P, KT, HW], BF16, kind="ExternalOutput")

    with tile.TileContext(nc) as tc:
        with (
            tc.tile_pool(name="xqp", bufs=2 * NCH) as xqp,
            tc.tile_pool(name="xbp", bufs=NCH + 2) as xbp,
            tc.tile_pool(name="wp", bufs=1) as wp,
            tc.tile_pool(name="actp", bufs=2) as actp,
            tc.tile_pool(name="deadp", bufs=1) as deadp,
            tc.tile_pool(name="thp", bufs=2) as thp,
            tc.tile_pool(name="smp", bufs=2) as smp,
            tc.tile_pool(name="erp", bufs=2 * MT) as erp,
            tc.tile_pool(name="chp", bufs=3) as chp,
            tc.tile_pool(name="tp", bufs=10) as tp,
            tc.tile_pool(name="otp", bufs=3) as otp,
            tc.tile_pool(name="psA", bufs=2, space="PSUM") as psA,
            tc.tile_pool(name="psB", bufs=6, space="PSUM") as psB,
        ):
            # ---- constants ----
            epst = wp.tile([P, 1], F32, name="epst", tag="epst")
            nc.vector.memset(epst[:], LN_EPS)

            # ---- weight tiles: one DMA per tensor, emitted in first-use
            # priority order interleaved with the sample-0 x loads ----
            wqr8_sb = wp.tile([P, KT, P], F8, name="wqr8sb", tag="wqr8sb")
            # wvr bf16 split per m-tile so vr(chunk0, m=0) only waits on 1/4
            # of the weight bytes at startup
            wvrb_sb = [
                wp.tile([P, KT, P], BF16, name=f"wvrbsb{m}", tag=f"wvrbsb{m}")
                for m in range(MT)
            ]
            wql_sb = wp.tile([P, KT, CH], F8, name="wqlsb", tag="wqlsb")
            wvl_sb = wp.tile([P, KT, CH], F8, name="wvlsb", tag="wvlsb")

            def emit_xq_load(s_, i_):
                t = xqp.tile([P, KT, NW], F8, name=f"xq{s_}_{i_}", tag="xq")
                nc.sync.dma_start(t[:], xq_d.ap()[s_, i_])
                return t

            def emit_xb_load(s_, i_):
                t = xbp.tile([P, KT, NW], BF16, name=f"xb{s_}_{i_}", tag="xb")
                nc.sync.dma_start(t[:], xb_d.ap()[s_, i_])
                return t

            # sample-0 loads up front, ordered so A(0) can start ASAP:
            # qr needs wqr8+xq(0,0); vr needs wvrb+xb(0,0); then stream the
            # rest chunk-by-chunk (A consumes xq(i)+xb(i) per chunk).
            nc.sync.dma_start(wqr8_sb[:], wqr8_d.ap()[:])
            xq_all = {0: [], 1: []}
            xb_all = {0: [], 1: []}
            xq_all[0].append(emit_xq_load(0, 0))
            nc.sync.dma_start(wvrb_sb[0][:], wvrb_d.ap()[:, :, 0:P])
            xb_all[0].append(emit_xb_load(0, 0))
            for m in range(1, MT):
                nc.sync.dma_start(
                    wvrb_sb[m][:], wvrb_d.ap()[:, :, m * P : (m + 1) * P]
                )
            for i in range(1, NCH):
                xq_all[0].append(emit_xq_load(0, i))
                xb_all[0].append(emit_xb_load(0, i))
                if i == 1:
                    nc.sync.dma_start(wql_sb[:], wql_d.ap()[:])
                elif i == 2:
                    nc.sync.dma_start(wvl_sb[:], wvl_d.ap()[:])

            def comp_mm(ps, w8, wr, xq, m, nk=2):
                """Residual-compensated DoubleRow conv into psum group.

                w8/wr: [P, KT, cols] fp8 tiles (wr=None for uncompensated);
                xq: [P, KT, NW] fp8; m: output m-tile index (cols slice).
                """
                lo, hi = m * P, (m + 1) * P
                wts = [w8] if wr is None else [w8, wr]
                n = len(wts) * (KT // nk)
                j = 0
                for wt in wts:
                    for a in range(KT // nk):
                        nc.tensor.matmul(
                            ps[:],
                            wt[:, nk * a : nk * a + nk, lo:hi],
                            xq[:, nk * a : nk * a + nk, :],
                            start=(j == 0), stop=(j == n - 1),
                            perf_mode=DR,
                        )
                        j += 1

            for s in range(S):
                xq_t = xq_all[s]

                # per-sample accumulators
                zpart = smp.tile([P, NCH], F32, name=f"zpart{s}", tag="zpart")
                ctxp = [
                    smp.tile([P, NCH], F32, name=f"ctxp{s}_{m}", tag=f"ctxp{m}")
                    for m in range(MT)
                ]
                gp = [
                    smp.tile([P, NCH], F32, name=f"gp{s}_{m}", tag=f"gp{m}")
                    for m in range(MT)
                ]

                # ---- phase A: qr conv (fp8 DR, uncompensated) + vr conv
                # (bf16 weights x bf16 xb tiles, near-exact) ----
                for i in range(NCH):
                    psq = psA.tile([P, NW], F32, name=f"psq{s}_{i}", tag="psA")
                    comp_mm(psq, wqr8_sb, None, xq_t[i], 0)
                    # exp(relu(z)) == max(exp(z), 1): ACT exp (1/SC unscale),
                    # then DVE in-place max with Z partials via accum
                    et = actp.tile([P, NW], F32, name=f"et{s}_{i}", tag="et")
                    nc.scalar.activation(et[:], psq[:], Act.Exp, scale=1.0 / SC)
                    nc.vector.tensor_scalar(
                        et[:], et[:], 1.0, 0.0, Alu.max, Alu.add,
                        accum_out=zpart[:, i : i + 1],
                    )
                    for m in range(MT):
                        psv = psB.tile([P, NW], F32, name=f"psv{s}a{i}_{m}", tag="psB")
                        for a in range(KT):
                            nc.tensor.matmul(
                                psv[:],
                                wvrb_sb[m][:, a, :],
                                xb_all[s][i][:, a, :],
                                start=(a == 0), stop=(a == KT - 1),
                            )
                        # ctx partial: sum_n relu(vr) * exp(relu(qr))
                        scr = deadp.tile([P, NW], F32, name=f"sttscr{s}", tag="sttscr")
                        nc.vector.scalar_tensor_tensor(
                            scr[:], psv[:], 0.0, et[:], Alu.max, Alu.mult,
                            accum_out=ctxp[m][:, i : i + 1],
                        )

                # ---- finalize mask Z and context; layernorm stats ----
                Zt = smp.tile([P, 1], F32, name=f"Z{s}", tag="Z")
                nc.vector.tensor_reduce(Zt[:], zpart[:], AxX, Alu.add)
                rZ = smp.tile([P, 1], F32, name=f"rZ{s}", tag="rZ")
                nc.vector.reciprocal(rZ[:], Zt[:])
                ctx44 = smp.tile([P, MT], F32, name=f"ctx44{s}", tag="ctx44")
                for m in range(MT):
                    cred = smp.tile([P, 1], F32, name=f"cred{s}_{m}", tag="cred")
                    nc.vector.tensor_reduce(cred[:], ctxp[m][:], AxX, Alu.add)
                    nc.vector.tensor_scalar(
                        ctx44[:, m : m + 1], cred[:], rZ[:], None, Alu.mult
                    )
                lnsum = smp.tile([P, MT], F32, name=f"lnsum{s}", tag="lnsum")
                nc.gpsimd.partition_all_reduce(
                    lnsum[:], ctx44[:], P, bass_isa.ReduceOp.add
                )
                tot = smp.tile([P, 1], F32, name=f"tot{s}", tag="tot")
                nc.vector.tensor_reduce(tot[:], lnsum[:], AxX, Alu.add)
                mu = smp.tile([P, 1], F32, name=f"mu{s}", tag="mu")
                nc.vector.tensor_scalar(mu[:], tot[:], 1.0 / CH, None, Alu.mult)
                d44 = smp.tile([P, MT], F32, name=f"d44{s}", tag="d44")
                nc.vector.tensor_scalar(d44[:], ctx44[:], mu[:], None, Alu.subtract)
                d2 = smp.tile([P, MT], F32, name=f"d2{s}", tag="d2")
                nc.vector.tensor_tensor(d2[:], d44[:], d44[:], Alu.mult)
                vsum = smp.tile([P, MT], F32, name=f"vsum{s}", tag="vsum")
                nc.gpsimd.partition_all_reduce(
                    vsum[:], d2[:], P, bass_isa.ReduceOp.add
                )
                vtot = smp.tile([P, 1], F32, name=f"vtot{s}", tag="vtot")
                nc.vector.tensor_reduce(vtot[:], vsum[:], AxX, Alu.add)
                var = smp.tile([P, 1], F32, name=f"var{s}", tag="var")
                nc.vector.tensor_scalar(var[:], vtot[:], 1.0 / CH, None, Alu.mult)

                # ---- phase B: ql conv, plain fp8 DoubleRow ----
                for i in range(NCH):
                    if s + 1 < S and i < 4:
                        # next sample's fp8 loads early (bufs=16 -> fresh slots)
                        xq_all[s + 1].append(emit_xq_load(s + 1, 2 * i))
                        xq_all[s + 1].append(emit_xq_load(s + 1, 2 * i + 1))
                    for m in range(MT):
                        psv = psB.tile([P, NW], F32, name=f"psv{s}b{i}_{m}", tag="psB")
                        comp_mm(psv, wql_sb, None, xq_t[i], m)
                        # relu + accumulate mean partials; alternate engines
                        if m % 2 == 0:
                            scr = deadp.tile([P, NW], F32, name=f"qlscr{s}", tag="qlscr")
                            nc.scalar.activation(
                                scr[:], psv[:], Act.Relu, accum_out=gp[m][:, i : i + 1]
                            )
                        else:
                            scr2 = deadp.tile([P, NW], F32, name=f"sttscr{s}b", tag="sttscr")
                            nc.vector.tensor_scalar(
                                scr2[:], psv[:], 0.0, 0.0, Alu.max, Alu.add,
                                accum_out=gp[m][:, i : i + 1],
                            )

                # ---- channel softmax + LN finalize (overlaps C's vl convs) ----
                g44 = smp.tile([P, MT], F32, name=f"g44{s}", tag="g44")
                for m in range(MT):
                    nc.vector.tensor_reduce(g44[:, m : m + 1], gp[m][:], AxX, Alu.add)
                e44 = smp.tile([P, MT], F32, name=f"e44{s}", tag="e44")
                nc.scalar.activation(e44[:], g44[:], Act.Exp, scale=1.0 / (HW * SC))
                std = smp.tile([P, 1], F32, name=f"std{s}", tag="std")
                nc.scalar.activation(std[:], var[:], Act.Sqrt, bias=epst[:])
                rstd = smp.tile([P, 1], F32, name=f"rstd{s}", tag="rstd")
                nc.vector.reciprocal(rstd[:], std[:])
                spre = smp.tile([P, MT], F32, name=f"spre{s}", tag="spre")
                nc.vector.tensor_scalar(
                    spre[:], ctx44[:], mu[:], rstd[:], Alu.subtract, Alu.mult
                )
                s44 = smp.tile([P, MT], F32, name=f"s44{s}", tag="s44")
                nc.scalar.activation(s44[:], spre[:], Act.Sigmoid)
                sp44 = smp.tile([P, MT], F32, name=f"sp44{s}", tag="sp44")
                nc.vector.tensor_scalar(sp44[:], s44[:], 1.0, None, Alu.add)

                ze = smp.tile([P, MT], F32, name=f"ze{s}", tag="ze")
                nc.gpsimd.partition_all_reduce(ze[:], e44[:], P, bass_isa.ReduceOp.add)
                zet = smp.tile([P, 1], F32, name=f"zet{s}", tag="zet")
                nc.vector.tensor_reduce(zet[:], ze[:], AxX, Alu.add)
                rZc = smp.tile([P, 1], F32, name=f"rZc{s}", tag="rZc")
                nc.vector.reciprocal(rZc[:], zet[:])
                erep = []
                for m in range(MT):
                    er = erp.tile([P, P], BF16, name=f"erep{s}_{m}", tag="erep")
                    # 1/SC compensates the fp8 scaling of wvl
                    nc.vector.tensor_scalar(
                        er[:], e44[:, m : m + 1].broadcast_to([P, P]),
                        1.0 / SC, None, Alu.mult,
                    )
                    erep.append(er)

                # ---- phase C: vl conv (one chunk ahead) -> chan attn ->
                # finale + store.  vl fp8 DoubleRow, chan fp32r replicated ----
                th_t = {}

                def emit_vl(i):
                    ths = []
                    for m in range(MT):
                        psv = psB.tile([P, NW], F32, name=f"psv{s}c{i}_{m}", tag="psB")
                        comp_mm(psv, wvl_sb, None, xq_t[i], m)
                        th = thp.tile([P, NW], BF16, name=f"th{s}_{i}_{m}", tag=f"th{m}")
                        nc.scalar.activation(th[:], psv[:], Act.Relu)
                        ths.append(th)
                    th_t[i] = ths

                emit_vl(0)
                for i in range(NCH):
                    if i + 1 < NCH:
                        emit_vl(i + 1)
                    pschan = psA.tile([P, NW], F32, name=f"psc{s}_{i}", tag="psA")
                    for m in range(MT):
                        nc.tensor.matmul(
                            pschan[:], erep[m][:], th_t[i][m][:],
                            start=(m == 0), stop=(m == MT - 1),
                            skip_group_check=True,
                        )
                    del th_t[i]
                    chant = chp.tile([P, NW], BF16, name=f"ch{s}_{i}", tag="chant")
                    nc.scalar.activation(chant[:], pschan[:], Act.Sigmoid, scale=rZc[:])
                    if s + 1 < S:
                        xb_all[s + 1].append(emit_xb_load(s + 1, i))
                    # finale: seq rows k<4: x*(1 + s*chan); par rows: x*(chan+1+s).
                    # bf16 end-to-end.  The attn tiles t are TS ops (4x on DVE
                    # in bf16; STT has no fast mode so it is avoided): 3 on
                    # Pool, 3+1 on DVE, 1 on ACT (relu(chan+sp) == chan+sp
                    # since chan,s > 0).  All 8 multiplies are 2x TT on DVE.
                    ot = otp.tile([P, KT, NW], BF16, name=f"ot{s}_{i}", tag="ot")
                    for k in range(KT):
                        xf = xb_all[s][i][:, k, :]
                        t = tp.tile([P, NW], BF16, name=f"t{s}_{i}_{k}", tag="t")
                        if k < MT:
                            # t = 1 + s*chan on DVE (4x bf16 TS)
                            nc.vector.tensor_scalar(
                                t[:], chant[:], s44[:, k : k + 1], 1.0,
                                Alu.mult, Alu.add,
                            )
                        elif k == MT:
                            # t = chan + (1+s) on DVE (4x TS; gpsimd's add form
                            # codegens ~6us/op so Pool is kept out of the finale)
                            nc.vector.tensor_scalar(
                                t[:], chant[:], sp44[:, k - MT : k - MT + 1], None,
                                Alu.add,
                            )
                        else:
                            nc.scalar.activation(
                                t[:], chant[:], Act.Relu,
                                bias=sp44[:, k - MT : k - MT + 1],
                            )
                        nc.vector.tensor_tensor(ot[:, k, :], t[:], xf, Alu.mult)
                    # one store per chunk: sbuf [P][KT, NW] -> dram (P, KT, NW)
                    nc.sync.dma_start(
                        out_d.ap()[s, :, :, i * NW : (i + 1) * NW], ot[:]
                    )

    nc.compile()
    return nc


def _prep_inputs(x, w_qr, w_vr, w_ql, w_vl):
    import ml_dtypes

    f8 = np.dtype(ml_dtypes.float8_e4m3)
    bf16 = np.dtype(ml_dtypes.bfloat16)

    x = np.asarray(x, dtype=np.float32).reshape(B, C, HW)
    wts = {}

    def pack_w(w):
        # (out, in) -> [P, KT, out]: w_pk[p, k, o] = w[o, 128k + p]
        w = np.asarray(w, dtype=np.float32)
        return np.ascontiguousarray(w.T.reshape(KT, P, CH).transpose(1, 0, 2))

    def comp8(w):
        w8 = (w * SC).astype(f8)
        r8 = ((w - w8.astype(np.float32) / SC) * SC).astype(f8)
        return w8, r8

    wts["wvrb"] = pack_w(w_vr).astype(bf16)
    wts["wql"] = (pack_w(w_ql) * SC).astype(f8)
    wts["wvl"] = (pack_w(w_vl) * SC).astype(f8)
    q = np.asarray(w_qr, dtype=np.float32).reshape(KT, P).T  # [P, KT]
    qrep = np.ascontiguousarray(np.broadcast_to(q[:, :, None], (P, KT, P)))
    wts["wqr8"] = (qrep * SC).astype(f8)

    in_maps = []
    for c in range(N_CORES):
        m = dict(wts)
        # [S, chunk, P, KT, NW]: xs[s, i, p, k, n] = x[s, 128k+p, 512i+n]
        xs = np.ascontiguousarray(
            x[S * c : S * (c + 1)]
            .reshape(S, KT, P, NCH, NW)
            .transpose(0, 3, 2, 1, 4)
        )
        m["xq"] = xs.astype(f8)
        m["xb"] = xs.astype(bf16)
        in_maps.append(m)
    return in_maps


def _run(x, w_qr, w_vr, w_ql, w_vl, trace=False):
    if "nc" not in _cache:
        _cache["nc"] = _build()
    nc = _cache["nc"]
    in_maps = _prep_inputs(x, w_qr, w_vr, w_ql, w_vl)
    res = bass_utils.run_bass_kernel_spmd(
        nc, in_maps, core_ids=list(range(N_CORES)), trace=trace
    )
    out = np.empty((B, C, HW), np.float32)
    for c in range(N_CORES):
        out[S * c : S * (c + 1)] = (
            res.results[c]["out"]
            .reshape(S, P, KT, HW)
            .transpose(0, 2, 1, 3)
            .reshape(S, C, HW)
            .astype(np.float32)
        )
    return out.reshape(B, C, H, W), res


def kernel(x, w_qr, w_vr, w_ql, w_vl):
    out, _ = _run(x, w_qr, w_vr, w_ql, w_vl, trace=False)
    return out


# revision 28
# speedup vs baseline: 2.0794x; 1.1873x over previous
"""Trainium2 Bass kernel for the dual-attention module (spatial + channel attention).

Contract: kernel(**inputs) takes the FULL inputs (x: (16,1024,64,64) f32 plus four
1x1-conv weight matrices) and returns the FULL output (16,1024,64,64) f32.
Internally shards data-parallel over batch across 8 NeuronCores (2 samples/core),
weights replicated.

Per-sample math (b, c=1024, ch=512, hw=4096):
  conv(w) = relu(w @ X)               X = x[b] as (1024, 4096)
  mask    = softmax(conv(w_qr))       over hw          (spatial attn branch)
  ctx     = conv(w_vr) @ mask         (ch,)
  s       = sigmoid(layernorm(ctx))   (ch,)
  avg     = softmax(mean_hw(conv(w_ql)))               (channel attn branch)
  chan    = sigmoid(avg @ conv(w_vl)) (hw,)
  out[0:512]    = x * (1 + s*chan)                     ("sequence")
  out[512:1024] = x * (1 + s + chan)                   ("parallel")

Kernel strategy per core (mixed bf16 / fp8-DoubleRow convs):
  - On TRN2 every 512-col matmul streams at ~219ns regardless of dtype, so
    instruction COUNT is what matters; fp8 DoubleRow contracts 2 k-tiles per
    instruction (2x instruction efficiency), bf16 contracts 1.
  - vr (context branch, layernorm-amplified) runs in bf16 from the xb tiles
    that are resident anyway for the finale multiply: same instruction count
    as residually-compensated fp8 but near-exact.  qr (mask logits, also
    LN-amplified but a single output row) runs uncompensated fp8 DoubleRow:
    its ~5e-3 error fits the budget freed by the bf16 vr.  ql/vl stay plain
    fp8 DoubleRow (their error washes out through mean(4096)/sum(512) and
    sigmoids).  Scale SC=256 on the fp8 weights folds out via exp scales.
  - x is loaded twice: fp8 (conv rhs, 4MB/sample) and bf16 (finale multiply,
    8MB/sample).  Output is stored bf16 (8MB/sample) and upconverted on the
    host; bf16 keeps elementwise error ~4e-3 << the 2e-2 gate.
  - Softmaxes are computed unnormalized (exp only); 1/Z folded into later
    scalar multiplies.  qr weights are column-replicated so mask psums arrive
    broadcast across partitions; the channel contraction uses a replicated
    fp32r lhsT so chan arrives pre-broadcast.
  - Cross-partition reductions (LN stats, channel-softmax Z) via exact-f32
    gpsimd.partition_all_reduce.
  - Phase C software-pipelines the vl conv one chunk ahead of the chan
    contraction so the PE never waits on the ACT relu or the post-B stats.
  - Finale ops are bf16 end-to-end (DVE 2x mode) and spread across
    Pool/DVE/ACT; stores on the Sync DGE with next-sample loads emitted ahead.
"""

import sys

sys.path.insert(0, "/opt/trn_rl_repo")

import numpy as np

import concourse.bass as bass  # noqa: F401  (bass must import before bacc)
import concourse.tile as tile
from concourse import bacc, bass_isa, bass_utils, mybir

# Problem constants (hardcoded per contract).
B, C, H, W = 16, 1024, 64, 64
HW = H * W               # 4096
CH = C // 2              # 512
N_CORES = 8
S = B // N_CORES         # 2 samples per core
P = 128                  # SBUF partitions
KT = C // P              # 8 k-tiles over input channels
MT = CH // P             # 4 m-tiles over output channels
NW = 512                 # n-chunk width (one PSUM bank of f32)
NCH = HW // NW           # 8 n-chunks
LN_EPS = 1e-5
SC = 256.0               # fp8 weight scale (folded out through exp scales / LN)

F32 = mybir.dt.float32
F32R = mybir.dt.float32r
BF16 = mybir.dt.bfloat16
F8 = mybir.dt.float8e4
Alu = mybir.AluOpType
Act = mybir.ActivationFunctionType
AxX = mybir.AxisListType.X
DR = mybir.MatmulPerfMode.DoubleRow

_cache = {}


def _build():
    nc = bacc.Bacc(
        "TRN2",
        target_bir_lowering=False,
        debug=False,
        num_devices=N_CORES,
        dynamic_dma_scratch_size=512,
    )

    # x: [S, chunk, P, KT, NW] so one chunk is a single DMA with contiguous
    # per-partition bytes (4KB fp8 / 8KB bf16); weights partition-major.
    xq_d = nc.dram_tensor("xq", [S, NCH, P, KT, NW], F8, kind="ExternalInput")
    xb_d = nc.dram_tensor("xb", [S, NCH, P, KT, NW], BF16, kind="ExternalInput")
    wqr8_d = nc.dram_tensor("wqr8", [P, KT, P], F8, kind="ExternalInput")
    wvrb_d = nc.dram_tensor("wvrb", [P, KT, CH], BF16, kind="ExternalInput")
    wql_d = nc.dram_tensor("wql", [P, KT, CH], F8, kind="ExternalInput")
    wvl_d = nc.dram_tensor("wvl", [P, KT, CH], F8, kind="ExternalInput")
    # out is [S, P, KT, HW] so a whole chunk stores as ONE dma whose dram AP
    # dims (P, KT, NW) match the sbuf tile; host transposes (cheap view).
    out_d = nc.dram_tensor("out", [S, P, KT, HW], BF16, kind="ExternalOutput")

    with tile.TileContext(nc) as tc:
        with (
            tc.tile_pool(name="xqp", bufs=2 * NCH) as xqp,
            tc.tile_pool(name="xbp", bufs=NCH + 2) as xbp,
            tc.tile_pool(name="wp", bufs=1) as wp,
            tc.tile_pool(name="actp", bufs=2) as actp,
            tc.tile_pool(name="deadp", bufs=1) as deadp,
            tc.tile_pool(name="thp", bufs=2) as thp,
            tc.tile_pool(name="smp", bufs=2) as smp,
            tc.tile_pool(name="erp", bufs=2 * MT) as erp,
            tc.tile_pool(name="chp", bufs=3) as chp,
            tc.tile_pool(name="tp", bufs=10) as tp,
            tc.tile_pool(name="otp", bufs=3) as otp,
            tc.tile_pool(name="psA", bufs=2, space="PSUM") as psA,
            tc.tile_pool(name="psB", bufs=6, space="PSUM") as psB,
        ):
            # ---- constants ----
            epst = wp.tile([P, 1], F32, name="epst", tag="epst")
            nc.vector.memset(epst[:], LN_EPS)

            # ---- weight tiles: one DMA per tensor, emitted in first-use
            # priority order interleaved with the sample-0 x loads ----
            wqr8_sb = wp.tile([P, KT, P], F8, name="wqr8sb", tag="wqr8sb")
            wvrb_sb = wp.tile([P, KT, CH], BF16, name="wvrbsb", tag="wvrbsb")
            wql_sb = wp.tile([P, KT, CH], F8, name="wqlsb", tag="wqlsb")
            wvl_sb = wp.tile([P, KT, CH], F8, name="wvlsb", tag="wvlsb")

            def emit_xq_load(s_, i_):
                t = xqp.tile([P, KT, NW], F8, name=f"xq{s_}_{i_}", tag="xq")
                nc.sync.dma_start(t[:], xq_d.ap()[s_, i_])
                return t

            def emit_xb_load(s_, i_):
                t = xbp.tile([P, KT, NW], BF16, name=f"xb{s_}_{i_}", tag="xb")
                nc.sync.dma_start(t[:], xb_d.ap()[s_, i_])
                return t

            # sample-0 loads up front, ordered so A(0) can start ASAP:
            # qr needs wqr8+xq(0,0); vr needs wvrb+xb(0,0); then stream the
            # rest chunk-by-chunk (A consumes xq(i)+xb(i) per chunk).
            nc.sync.dma_start(wqr8_sb[:], wqr8_d.ap()[:])
            xq_all = {0: [], 1: []}
            xb_all = {0: [], 1: []}
            xq_all[0].append(emit_xq_load(0, 0))
            nc.sync.dma_start(wvrb_sb[:], wvrb_d.ap()[:])
            xb_all[0].append(emit_xb_load(0, 0))
            for i in range(1, NCH):
                xq_all[0].append(emit_xq_load(0, i))
                xb_all[0].append(emit_xb_load(0, i))
                if i == 1:
                    nc.sync.dma_start(wql_sb[:], wql_d.ap()[:])
                elif i == 2:
                    nc.sync.dma_start(wvl_sb[:], wvl_d.ap()[:])

            def comp_mm(ps, w8, wr, xq, m, nk=2):
                """Residual-compensated DoubleRow conv into psum group.

                w8/wr: [P, KT, cols] fp8 tiles (wr=None for uncompensated);
                xq: [P, KT, NW] fp8; m: output m-tile index (cols slice).
                """
                lo, hi = m * P, (m + 1) * P
                wts = [w8] if wr is None else [w8, wr]
                n = len(wts) * (KT // nk)
                j = 0
                for wt in wts:
                    for a in range(KT // nk):
                        nc.tensor.matmul(
                            ps[:],
                            wt[:, nk * a : nk * a + nk, lo:hi],
                            xq[:, nk * a : nk * a + nk, :],
                            start=(j == 0), stop=(j == n - 1),
                            perf_mode=DR,
                        )
                        j += 1

            for s in range(S):
                xq_t = xq_all[s]

                # per-sample accumulators
                zpart = smp.tile([P, NCH], F32, name=f"zpart{s}", tag="zpart")
                ctxp = [
                    smp.tile([P, NCH], F32, name=f"ctxp{s}_{m}", tag=f"ctxp{m}")
                    for m in range(MT)
                ]
                gp = [
                    smp.tile([P, NCH], F32, name=f"gp{s}_{m}", tag=f"gp{m}")
                    for m in range(MT)
                ]

                # ---- phase A: qr conv (fp8 DR, uncompensated) + vr conv
                # (bf16 weights x bf16 xb tiles, near-exact) ----
                for i in range(NCH):
                    psq = psA.tile([P, NW], F32, name=f"psq{s}_{i}", tag="psA")
                    comp_mm(psq, wqr8_sb, None, xq_t[i], 0)
                    # exp(relu(z)) == max(exp(z), 1): ACT exp (1/SC unscale),
                    # then DVE in-place max with Z partials via accum
                    et = actp.tile([P, NW], F32, name=f"et{s}_{i}", tag="et")
                    nc.scalar.activation(et[:], psq[:], Act.Exp, scale=1.0 / SC)
                    nc.vector.tensor_scalar(
                        et[:], et[:], 1.0, 0.0, Alu.max, Alu.add,
                        accum_out=zpart[:, i : i + 1],
                    )
                    for m in range(MT):
                        psv = psB.tile([P, NW], F32, name=f"psv{s}a{i}_{m}", tag="psB")
                        lo, hi = m * P, (m + 1) * P
                        for a in range(KT):
                            nc.tensor.matmul(
                                psv[:],
                                wvrb_sb[:, a, lo:hi],
                                xb_all[s][i][:, a, :],
                                start=(a == 0), stop=(a == KT - 1),
                            )
                        # ctx partial: sum_n relu(vr) * exp(relu(qr))
                        scr = deadp.tile([P, NW], F32, name=f"sttscr{s}", tag="sttscr")
                        nc.vector.scalar_tensor_tensor(
                            scr[:], psv[:], 0.0, et[:], Alu.max, Alu.mult,
                            accum_out=ctxp[m][:, i : i + 1],
                        )

                # ---- finalize mask Z and context; layernorm stats ----
                Zt = smp.tile([P, 1], F32, name=f"Z{s}", tag="Z")
                nc.vector.tensor_reduce(Zt[:], zpart[:], AxX, Alu.add)
                rZ = smp.tile([P, 1], F32, name=f"rZ{s}", tag="rZ")
                nc.vector.reciprocal(rZ[:], Zt[:])
                ctx44 = smp.tile([P, MT], F32, name=f"ctx44{s}", tag="ctx44")
                for m in range(MT):
                    cred = smp.tile([P, 1], F32, name=f"cred{s}_{m}", tag="cred")
                    nc.vector.tensor_reduce(cred[:], ctxp[m][:], AxX, Alu.add)
                    nc.vector.tensor_scalar(
                        ctx44[:, m : m + 1], cred[:], rZ[:], None, Alu.mult
                    )
                lnsum = smp.tile([P, MT], F32, name=f"lnsum{s}", tag="lnsum")
                nc.gpsimd.partition_all_reduce(
                    lnsum[:], ctx44[:], P, bass_isa.ReduceOp.add
                )
                tot = smp.tile([P, 1], F32, name=f"tot{s}", tag="tot")
                nc.vector.tensor_reduce(tot[:], lnsum[:], AxX, Alu.add)
                mu = smp.tile([P, 1], F32, name=f"mu{s}", tag="mu")
                nc.vector.tensor_scalar(mu[:], tot[:], 1.0 / CH, None, Alu.mult)
                d44 = smp.tile([P, MT], F32, name=f"d44{s}", tag="d44")
                nc.vector.tensor_scalar(d44[:], ctx44[:], mu[:], None, Alu.subtract)
                d2 = smp.tile([P, MT], F32, name=f"d2{s}", tag="d2")
                nc.vector.tensor_tensor(d2[:], d44[:], d44[:], Alu.mult)
                vsum = smp.tile([P, MT], F32, name=f"vsum{s}", tag="vsum")
                nc.gpsimd.partition_all_reduce(
                    vsum[:], d2[:], P, bass_isa.ReduceOp.add
                )
                vtot = smp.tile([P, 1], F32, name=f"vtot{s}", tag="vtot")
                nc.vector.tensor_reduce(vtot[:], vsum[:], AxX, Alu.add)
                var = smp.tile([P, 1], F32, name=f"var{s}", tag="var")
                nc.vector.tensor_scalar(var[:], vtot[:], 1.0 / CH, None, Alu.mult)

                # ---- phase B: ql conv, plain fp8 DoubleRow ----
                for i in range(NCH):
                    if s + 1 < S and i < 4:
                        # next sample's fp8 loads early (bufs=16 -> fresh slots)
                        xq_all[s + 1].append(emit_xq_load(s + 1, 2 * i))
                        xq_all[s + 1].append(emit_xq_load(s + 1, 2 * i + 1))
                    for m in range(MT):
                        psv = psB.tile([P, NW], F32, name=f"psv{s}b{i}_{m}", tag="psB")
                        comp_mm(psv, wql_sb, None, xq_t[i], m)
                        # relu + accumulate mean partials; alternate engines
                        if m % 2 == 0:
                            scr = deadp.tile([P, NW], F32, name=f"qlscr{s}", tag="qlscr")
                            nc.scalar.activation(
                                scr[:], psv[:], Act.Relu, accum_out=gp[m][:, i : i + 1]
                            )
                        else:
                            scr2 = deadp.tile([P, NW], F32, name=f"sttscr{s}b", tag="sttscr")
                            nc.vector.tensor_scalar(
                                scr2[:], psv[:], 0.0, 0.0, Alu.max, Alu.add,
                                accum_out=gp[m][:, i : i + 1],
                            )

                # ---- channel softmax + LN finalize (overlaps C's vl convs) ----
                g44 = smp.tile([P, MT], F32, name=f"g44{s}", tag="g44")
                for m in range(MT):
                    nc.vector.tensor_reduce(g44[:, m : m + 1], gp[m][:], AxX, Alu.add)
                e44 = smp.tile([P, MT], F32, name=f"e44{s}", tag="e44")
                nc.scalar.activation(e44[:], g44[:], Act.Exp, scale=1.0 / (HW * SC))
                std = smp.tile([P, 1], F32, name=f"std{s}", tag="std")
                nc.scalar.activation(std[:], var[:], Act.Sqrt, bias=epst[:])
                rstd = smp.tile([P, 1], F32, name=f"rstd{s}", tag="rstd")
                nc.vector.reciprocal(rstd[:], std[:])
                spre = smp.tile([P, MT], F32, name=f"spre{s}", tag="spre")
                nc.vector.tensor_scalar(
                    spre[:], ctx44[:], mu[:], rstd[:], Alu.subtract, Alu.mult
                )
                s44 = smp.tile([P, MT], F32, name=f"s44{s}", tag="s44")
                nc.scalar.activation(s44[:], spre[:], Act.Sigmoid)
                sp44 = smp.tile([P, MT], F32, name=f"sp44{s}", tag="sp44")
                nc.vector.tensor_scalar(sp44[:], s44[:], 1.0, None, Alu.add)

                ze = smp.tile([P, MT], F32, name=f"ze{s}", tag="ze")
                nc.gpsimd.partition_all_reduce(ze[:], e44[:], P, bass_isa.ReduceOp.add)
                zet = smp.tile([P, 1], F32, name=f"zet{s}", tag="zet")
                nc.vector.tensor_reduce(zet[:], ze[:], AxX, Alu.add)
                rZc = smp.tile([P, 1], F32, name=f"rZc{s}", tag="rZc")
                nc.vector.reciprocal(rZc[:], zet[:])
                erep = []
                for m in range(MT):
                    er = erp.tile([P, P], BF16, name=f"erep{s}_{m}", tag="erep")
                    # 1/SC compensates the fp8 scaling of wvl
                    nc.vector.tensor_scalar(
                        er[:], e44[:, m : m + 1].broadcast_to([P, P]),
                        1.0 / SC, None, Alu.mult,
                    )
                    erep.append(er)

                # ---- phase C: vl conv (one chunk ahead) -> chan attn ->
                # finale + store.  vl fp8 DoubleRow, chan fp32r replicated ----
                th_t = {}

                def emit_vl(i):
                    ths = []
                    for m in range(MT):
                        psv = psB.tile([P, NW], F32, name=f"psv{s}c{i}_{m}", tag="psB")
                        comp_mm(psv, wvl_sb, None, xq_t[i], m)
                        th = thp.tile([P, NW], BF16, name=f"th{s}_{i}_{m}", tag=f"th{m}")
                        nc.scalar.activation(th[:], psv[:], Act.Relu)
                        ths.append(th)
                    th_t[i] = ths

                emit_vl(0)
                for i in range(NCH):
                    if i + 1 < NCH:
                        emit_vl(i + 1)
                    pschan = psA.tile([P, NW], F32, name=f"psc{s}_{i}", tag="psA")
                    for m in range(MT):
                        nc.tensor.matmul(
                            pschan[:], erep[m][:], th_t[i][m][:],
                            start=(m == 0), stop=(m == MT - 1),
                            skip_group_check=True,
                        )
                    del th_t[i]
                    chant = chp.tile([P, NW], BF16, name=f"ch{s}_{i}", tag="chant")
                    nc.scalar.activation(chant[:], pschan[:], Act.Sigmoid, scale=rZc[:])
                    if s + 1 < S:
                        xb_all[s + 1].append(emit_xb_load(s + 1, i))
                    # finale: seq rows k<4: x*(1 + s*chan); par rows: x*(chan+1+s).
                    # bf16 end-to-end.  The attn tiles t are TS ops (4x on DVE
                    # in bf16; STT has no fast mode so it is avoided): 3 on
                    # Pool, 3+1 on DVE, 1 on ACT (relu(chan+sp) == chan+sp
                    # since chan,s > 0).  All 8 multiplies are 2x TT on DVE.
                    ot = otp.tile([P, KT, NW], BF16, name=f"ot{s}_{i}", tag="ot")
                    for k in range(KT):
                        xf = xb_all[s][i][:, k, :]
                        t = tp.tile([P, NW], BF16, name=f"t{s}_{i}_{k}", tag="t")
                        if k < MT:
                            # t = 1 + s*chan on DVE (4x bf16 TS)
                            nc.vector.tensor_scalar(
                                t[:], chant[:], s44[:, k : k + 1], 1.0,
                                Alu.mult, Alu.add,
                            )
                        elif k < KT - 1:
                            # t = chan + (1+s) on DVE (4x TS; gpsimd's add form
                            # codegens ~6us/op so Pool is kept out of the finale)
                            nc.vector.tensor_scalar(
                                t[:], chant[:], sp44[:, k - MT : k - MT + 1], None,
                                Alu.add,
                            )
                        else:
                            nc.scalar.activation(
                                t[:], chant[:], Act.Relu,
                                bias=sp44[:, k - MT : k - MT + 1],
                            )
                        nc.vector.tensor_tensor(ot[:, k, :], t[:], xf, Alu.mult)
                    # one store per chunk: sbuf [P][KT, NW] -> dram (P, KT, NW)
                    nc.sync.dma_start(
                        out_d.ap()[s, :, :, i * NW : (i + 1) * NW], ot[:]
                    )

    nc.compile()
    return nc


def _prep_inputs(x, w_qr, w_vr, w_ql, w_vl):
    import ml_dtypes

    f8 = np.dtype(ml_dtypes.float8_e4m3)
    bf16 = np.dtype(ml_dtypes.bfloat16)

    x = np.asarray(x, dtype=np.float32).reshape(B, C, HW)
    wts = {}

    def pack_w(w):
        # (out, in) -> [P, KT, out]: w_pk[p, k, o] = w[o, 128k + p]
        w = np.asarray(w, dtype=np.float32)
        return np.ascontiguousarray(w.T.reshape(KT, P, CH).transpose(1, 0, 2))

    def comp8(w):
        w8 = (w * SC).astype(f8)
        r8 = ((w - w8.astype(np.float32) / SC) * SC).astype(f8)
        return w8, r8

    wts["wvrb"] = pack_w(w_vr).astype(bf16)
    wts["wql"] = (pack_w(w_ql) * SC).astype(f8)
    wts["wvl"] = (pack_w(w_vl) * SC).astype(f8)
    q = np.asarray(w_qr, dtype=np.float32).reshape(KT, P).T  # [P, KT]
    qrep = np.ascontiguousarray(np.broadcast_to(q[:, :, None], (P, KT, P)))
    wts["wqr8"] = (qrep * SC).astype(f8)

    in_maps = []
    for c in range(N_CORES):
        m = dict(wts)
        # [S, chunk, P, KT, NW]: xs[s, i, p, k, n] = x[s, 128k+p, 512i+n]
        xs = np.ascontiguousarray(
            x[S * c : S * (c + 1)]
            .reshape(S, KT, P, NCH, NW)
            .transpose(0, 3, 2, 1, 4)
        )
        m["xq"] = xs.astype(f8)
        m["xb"] = xs.astype(bf16)
        in_maps.append(m)
    return in_maps


def _run(x, w_qr, w_vr, w_ql, w_vl, trace=False):
    if "nc" not in _cache:
        _cache["nc"] = _build()
    nc = _cache["nc"]
    in_maps = _prep_inputs(x, w_qr, w_vr, w_ql, w_vl)
    res = bass_utils.run_bass_kernel_spmd(
        nc, in_maps, core_ids=list(range(N_CORES)), trace=trace
    )
    out = np.empty((B, C, HW), np.float32)
    for c in range(N_CORES):
        out[S * c : S * (c + 1)] = (
            res.results[c]["out"]
            .reshape(S, P, KT, HW)
            .transpose(0, 2, 1, 3)
            .reshape(S, C, HW)
            .astype(np.float32)
        )
    return out.reshape(B, C, H, W), res


def kernel(x, w_qr, w_vr, w_ql, w_vl):
    out, _ = _run(x, w_qr, w_vr, w_ql, w_vl, trace=False)
    return out
